# revision 1
# baseline (speedup 1.0000x reference)
# 2D DCT-II [4096,4096] fp32 on 8 NeuronCores — v2 "folded dense".
#
# DCT even/odd fold: C[i, M-1-r] = (-1)^i C[i, r]  =>
#   U[2i'']   = sum_{r<2048} C[2i'', r]   * (X[r] + X[4095-r])
#   U[2i''+1] = sum_{r<2048} C[2i''+1, r] * (X[r] - X[4095-r])
# halving matmul work per pass. Orientation: data tiles are lhsT (stationary),
# cos-weights are rhs (moving), so pass-1 emits U^T tiles [c-part, i-free] and
# pass-2 consumes them directly after the AllToAll with no on-chip transposes.
# Intermediate rows travel in "folded" order [even-2048 || odd-2048]; the final
# store un-permutes via strided row/col addressing.
import numpy as np
from einops import rearrange
import concourse.bacc as bacc
import concourse.tile as tile
import concourse.mybir as mybir
from concourse import bass_utils

M = N = 4096
NC = 8
CB = 512          # columns per core (pass 1) / rows per core (pass 2)
KH = M // 2       # 2048 folded contraction length
KT = KH // 128    # 16 K-tiles
NCH = KH // 512   # 4 N-chunks of 512 per block

_BUILT = {}


def build_nc(repeat=1):
    dt = mybir.dt
    f32r = dt.float32r
    nc = bacc.Bacc("TRN2", target_bir_lowering=False, debug=False, num_devices=NC)

    xf = nc.dram_tensor("xf", [128, KT, CB], f32r, kind="ExternalInput")  # X[r<2048, cols]
    xr = nc.dram_tensor("xr", [128, KT, CB], f32r, kind="ExternalInput")  # X[4095-r, cols]
    we = nc.dram_tensor("we", [128, KT, KH], f32r, kind="ExternalInput")  # C[2k, r'].T
    wo = nc.dram_tensor("wo", [128, KT, KH], f32r, kind="ExternalInput")  # C[2k+1, r'].T
    y = nc.dram_tensor("y", [CB, N], f32r, kind="ExternalOutput")         # Y[rows_mine] true order

    with tile.TileContext(nc) as tc:
        with tc.tile_pool(name="dram", bufs=1, space="DRAM") as dram:
            z1 = dram.tile([NC, CB, CB], f32r)  # [chunk, c-local, folded-row]
            z2 = dram.tile([NC, CB, CB], f32r)
            for _rep in range(repeat):
                # ================= pass 1 =================
                with (
                    tc.tile_pool(name="xstage", bufs=3) as xstage,
                    tc.tile_pool(name="xfold", bufs=1) as xfold,
                    tc.tile_pool(name="wsl", bufs=2) as wsl,
                    tc.tile_pool(name="ps1", bufs=4, space="PSUM") as ps1,
                    tc.tile_pool(name="ev1", bufs=3) as ev1,
                ):
                    xp = xfold.tile([128, KT, CB], f32r, tag="xp")
                    xm = xfold.tile([128, KT, CB], f32r, tag="xm")
                    for k in range(KT):
                        tf = xstage.tile([128, CB], f32r, tag="tf")
                        tr = xstage.tile([128, CB], f32r, tag="tr")
                        nc.sync.dma_start(out=tf[:], in_=xf[:, k])
                        nc.sync.dma_start(out=tr[:], in_=xr[:, k])
                        nc.vector.tensor_add(xp[:, k], tf[:], tr[:])
                        nc.vector.tensor_sub(xm[:, k], tf[:], tr[:])
                    for blk, (xb, wb) in enumerate(((xp, we), (xm, wo))):
                        for nch in range(NCH):
                            wt = wsl.tile([128, KT, 512], f32r, tag="wslab")
                            nc.sync.dma_start(out=wt[:], in_=wb[:, :, nch * 512:(nch + 1) * 512])
                            for cm in range(CB // 128):
                                psum = ps1.tile([128, 512], dt.float32, tag="ps")
                                for k in range(KT):
                                    nc.tensor.matmul(psum[:], xb[:, k, cm * 128:(cm + 1) * 128],
                                                     wt[:, k],
                                                     start=(k == 0), stop=(k == KT - 1))
                                ev = ev1.tile([128, 512], f32r, tag="ev")
                                nc.vector.tensor_copy(ev[:], psum[:])
                                for piece in range(2):
                                    ch = nch * 2 + piece
                                    fold0 = blk * 256
                                    nc.sync.dma_start(
                                        out=z1[ch, cm * 128:(cm + 1) * 128,
                                               fold0:fold0 + 256],
                                        in_=ev[:, piece * 256:(piece + 1) * 256])

                # ================= A2A =================
                nc.gpsimd.collective_compute(
                    "AllToAll", mybir.AluOpType.bypass,
                    replica_groups=[list(range(NC))],
                    ins=[z1[:].opt()], outs=[z2[:].opt()])

                # ================= pass 2 =================
                with (
                    tc.tile_pool(name="zstage", bufs=3) as zstage,
                    tc.tile_pool(name="zfold", bufs=1) as zfold,
                    tc.tile_pool(name="wsl2", bufs=2) as wsl2,
                    tc.tile_pool(name="ps2", bufs=4, space="PSUM") as ps2,
                    tc.tile_pool(name="yt", bufs=1) as ytp,
                ):
                    zp = zfold.tile([128, KT, CB], f32r, tag="zp")
                    zm = zfold.tile([128, KT, CB], f32r, tag="zm")
                    z2f = z2[:].rearrange("s c r -> (s c) r")
                    for kt in range(KT):
                        tf = zstage.tile([128, CB], f32r, tag="tf2")
                        tr = zstage.tile([128, CB], f32r, tag="tr2")
                        # forward cols: c = 128*kt + p
                        nc.sync.dma_start(out=tf[:], in_=z2f[kt * 128:(kt + 1) * 128, :])
                        # reversed cols: c = 4095 - (128*kt + p)
                        nc.sync.dma_start(out=tr[:],
                                          in_=z2f[4095 - kt * 128:4095 - (kt + 1) * 128:-1, :])
                        nc.vector.tensor_add(zp[:, kt], tf[:], tr[:])
                        nc.vector.tensor_sub(zm[:, kt], tf[:], tr[:])
                    ytiles = []
                    for rm in range(CB // 128):
                        ytl = ytp.tile([128, N], f32r, tag=f"yt{rm}")
                        ytiles.append(ytl)
                    for blk, (zb, wb) in enumerate(((zp, we), (zm, wo))):
                        for nch in range(NCH):
                            wt = wsl2.tile([128, KT, 512], f32r, tag="wslab2")
                            nc.sync.dma_start(out=wt[:], in_=wb[:, :, nch * 512:(nch + 1) * 512])
                            for rm in range(CB // 128):
                                psum = ps2.tile([128, 512], dt.float32, tag="ps2")
                                for k in range(KT):
                                    nc.tensor.matmul(psum[:], zb[:, k, rm * 128:(rm + 1) * 128],
                                                     wt[:, k],
                                                     start=(k == 0), stop=(k == KT - 1))
                                # k-true = 2*(nch*512 + k'') + blk  -> strided copy
                                dst = ytiles[rm][:].rearrange("p (a b) -> p a b", b=2)
                                nc.vector.tensor_copy(
                                    dst[:, nch * 512:(nch + 1) * 512, blk], psum[:])
                    # final store: folded row-pos -> true rows
                    # rm 0/1: fp=rm*128+p -> true = 2*fp ; rm 2/3: true = 2*(fp-256)+1
                    yv = y[:].rearrange("(a b) n -> a b n", b=2)  # [2048, 2, 4096]
                    for rm in range(CB // 128):
                        if rm < 2:
                            out_ap = yv[rm * 128:(rm + 1) * 128, 0, :]
                        else:
                            out_ap = yv[(rm - 2) * 128:(rm - 1) * 128, 1, :]
                        nc.sync.dma_start(out=out_ap, in_=ytiles[rm][:])

    nc.compile()
    return nc


def _weights():
    n = np.arange(M, dtype=np.float64)
    k = np.arange(M, dtype=np.float64)
    C = np.cos(np.pi * (2.0 * n[None, :] + 1.0) * k[:, None] / (2.0 * M))
    We = np.ascontiguousarray(C[0::2, :KH].T).astype(np.float32)  # [r', i'']
    Wo = np.ascontiguousarray(C[1::2, :KH].T).astype(np.float32)
    return We, Wo


def tile3(a):
    return np.ascontiguousarray(rearrange(a, "(m p) n -> p m n", p=128))


def kernel(x, expkM=None, expkN=None, trace=False):
    x = np.asarray(x, dtype=np.float32)
    if "nc" not in _BUILT:
        _BUILT["nc"] = build_nc()
        We, Wo = _weights()
        _BUILT["we"] = tile3(We)
        _BUILT["wo"] = tile3(Wo)
    nc = _BUILT["nc"]
    xrev = x[::-1, :]
    in_maps = []
    for c in range(NC):
        sl = slice(c * CB, (c + 1) * CB)
        in_maps.append({
            "xf": tile3(x[:KH, sl]),
            "xr": tile3(xrev[:KH, sl]),
            "we": _BUILT["we"],
            "wo": _BUILT["wo"],
        })
    res = bass_utils.run_bass_kernel_spmd(nc, in_maps, core_ids=list(range(NC)),
                                          trace=trace)
    _BUILT["last_res"] = res
    out = np.concatenate([res.results[c]["y"] for c in range(NC)], axis=0)
    return out.astype(np.float32)
